# revision 24
# baseline (speedup 1.0000x reference)
"""Trainium2 kernel for nn_M3oE: multi-domain MoE over 26 categorical embeddings.

Sharding: data-parallel over batch across 8 NeuronCores (2048 rows each),
embedding tables replicated in DRAM (gathered directly from HBM via
indirect DMA with inline f32->bf16 cast; tables never enter SBUF).

The kernel is gather-stream-bound: the SWDGE indirect DMA processes exactly
one offset per partition per instruction (HW-probed; multi-offset forms are
UB on the built-in Q7 ucode, and the custom gather ucode library is not
shipped in this image), so the 53248 embedding-row fetches per core cost
416 instructions at ~1.4us each (~1.09us Q7 ucode + ~0.31us issue gap)
= ~582us, regardless of batching. All compute hides under that stream; the
span beyond it is just NEFF startup (~10us) and the last group's compute
tail, which variable-size tile groups (4,4,4,3,1) keep small.

Per-core pipeline (matmuls in bf16, PSUM f32):
  1. indirect-DMA gather of 26 rows/sample into xg [128, tiles*416] bf16
  2. PE transpose of x chunks -> xT [416(k-chunks), cols] (DVE evict, bf16)
  3. 8 experts: h1T = relu(W1^T xT + b1) (ACT evict fuses bias+relu+cast);
     h2T = relu(W2^T h1T + b2) (DVE add+max, off ACT's critical path);
     s_e = Wo . h2T accumulated into one [8, cols] PSUM tile via
     zero-padded Wo blocks
  4. domain-gated softmax: glog = Wg^T xT ; exp via ACT (+bg bias);
     one-hot domain mask (host-built) picks the sample's domain; expert-sum
     matmul (sel8) compresses to [8, cols]; denominator via ones32 matmul
  5. logits = (sum_e gsel_e * s_e) / denom + bo   (division via DVE recip)
"""

import ml_dtypes
import numpy as np

import concourse.bacc as bacc
import concourse.mybir as mybir
import concourse.tile as tile
from concourse.bass import IndirectOffsetOnAxis
from concourse.bass_utils import run_bass_kernel_spmd

F = 26
V = 100000
DK = 16
D = 4
E = 8
H1 = 128
H2 = 64
B = 16384
IN = F * DK  # 416
N_CORES = 8
PC = B // N_CORES  # 2048 rows per core
NT = PC // 128  # 16 batch-tiles of 128
NTG = 4  # tile groups (legacy; see GROUPS)
TGW = 512  # max columns per tile group (one PSUM bank of f32)
TPG = NT // NTG  # legacy
# Variable-size tile groups: big groups while the gather stream dominates,
# small trailing groups to shrink the post-stream compute tail.
GROUPS = [(0, 4), (4, 4), (8, 4), (12, 3), (15, 1)]
KCH = [(0, 128), (128, 128), (256, 128), (384, 32)]  # k-chunks of IN=416

F32 = mybir.dt.float32
F32R = mybir.dt.float32r
BF16 = mybir.dt.bfloat16
I32 = mybir.dt.int32

_cache = {}

# test-harness knobs (unused when the harness calls kernel() directly)
TRACE = False
TRACE_TMPDIR = None
LAST_RESULT = None


def _build(bo_val: float):
    nc = bacc.Bacc("TRN2", target_bir_lowering=False, debug=False,
                   num_devices=N_CORES)

    emb = nc.dram_tensor("emb_flat", [F * V, DK], BF16, kind="ExternalInput")
    idx = nc.dram_tensor("idx", [128, NT * F], I32, kind="ExternalInput")
    w1k = [nc.dram_tensor(f"w1k{i}", [w, E * H1], BF16, kind="ExternalInput")
           for i, (_, w) in enumerate(KCH)]
    w2c = nc.dram_tensor("w2c", [H1, E * H2], BF16, kind="ExternalInput")
    wo8 = nc.dram_tensor("wo8", [H2, E * 8], BF16, kind="ExternalInput")
    wgk = [nc.dram_tensor(f"wgk{i}", [w, D * E], BF16, kind="ExternalInput")
           for i, (_, w) in enumerate(KCH)]
    sel8 = nc.dram_tensor("sel8", [D * E, 8], BF16, kind="ExternalInput")
    ones8 = nc.dram_tensor("ones8", [E, 1], BF16, kind="ExternalInput")
    ones32 = nc.dram_tensor("ones32", [D * E, 1], BF16, kind="ExternalInput")
    b1t = nc.dram_tensor("b1t", [H1, E], F32, kind="ExternalInput")
    b2t = nc.dram_tensor("b2t", [H2, E], F32, kind="ExternalInput")
    bgc = nc.dram_tensor("bgc", [D * E, 1], F32, kind="ExternalInput")
    oh = nc.dram_tensor("oh", [D * E, PC], F32, kind="ExternalInput")
    id128 = nc.dram_tensor("id128", [128, 128], BF16, kind="ExternalInput")
    out = nc.dram_tensor("out", [1, PC], F32, kind="ExternalOutput")

    with tile.TileContext(nc) as tc:
        with (
            tc.tile_pool(name="const", bufs=1) as cpool,
            tc.tile_pool(name="xg", bufs=4) as xgpool,
            tc.tile_pool(name="xts", bufs=2 * len(KCH)) as xtspool,
            tc.tile_pool(name="h1s", bufs=3) as h1spool,
            tc.tile_pool(name="h2s", bufs=2) as h2spool,
            tc.tile_pool(name="gsb", bufs=2) as gsbpool,
            tc.tile_pool(name="fin", bufs=2) as finpool,
            tc.tile_pool(name="xtp", bufs=2, space="PSUM") as xtppool,
            tc.tile_pool(name="h1p", bufs=2, space="PSUM") as h1ppool,
            tc.tile_pool(name="h2p", bufs=1, space="PSUM") as h2ppool,
            tc.tile_pool(name="glp", bufs=1, space="PSUM") as glppool,
            tc.tile_pool(name="spp", bufs=1, space="PSUM") as sppool,
        ):
            # --- load constants ---
            def cload(dram, shape, dtype=None):
                t = cpool.tile(shape, dtype or dram.dtype, tag=dram.name)
                nc.sync.dma_start(out=t[:], in_=dram[:])
                return t

            idx_sb = cload(idx, [128, NT * F], I32)
            w1_sb = [cload(w1k[i], [w, E * H1]) for i, (_, w) in enumerate(KCH)]
            w2_sb = cload(w2c, [H1, E * H2])
            wo_sb = cload(wo8, [H2, E * 8])
            wg_sb = [cload(wgk[i], [w, D * E]) for i, (_, w) in enumerate(KCH)]
            sel_sb = cload(sel8, [D * E, 8])
            on8_sb = cload(ones8, [E, 1])
            on32_sb = cload(ones32, [D * E, 1])
            b1_sb = cload(b1t, [H1, E])
            b2_sb = cload(b2t, [H2, E])
            bg_sb = cload(bgc, [D * E, 1])
            oh_sb = cload(oh, [D * E, PC])
            id_sb = cload(id128, [128, 128])

            for t0, ntl in GROUPS:
                gw = ntl * 128  # columns in this group
                # --- gather: this group's embedding rows ---
                # SWDGE contract (HW-probed): one offset per partition per
                # instruction, offset read from the offset AP's first column
                # at each partition. F instructions per batch-tile is the
                # floor (~1.4us each: ~1.09us Q7 ucode + ~0.31us issue gap).
                xg = xgpool.tile([128, TPG * IN], BF16, tag="xg")
                for j in range(ntl * F):
                    jj = t0 * F + j
                    nc.gpsimd.indirect_dma_start(
                        out=xg[:, j * DK:(j + 1) * DK],
                        out_offset=None,
                        in_=emb[:],
                        in_offset=IndirectOffsetOnAxis(
                            ap=idx_sb[:, jj:jj + 1], axis=0),
                    )

                # --- transpose x -> xT per k-chunk (PE), evict to SBUF ---
                # For the last 1-tile group, defer the kc3 transpose (which
                # waits on the very last gather) so it doesn't head-of-line
                # block the kc0-2 expert matmuls on the in-order PE queue.
                def emit_chunk(kc):
                    koff, kw = KCH[kc]
                    xtp = xtppool.tile([128, TGW], BF16, space="PSUM",
                                       name=f"xtp{kc}", tag="xtp")
                    for tl in range(ntl):
                        nc.tensor.transpose(
                            out=xtp[0:kw, tl * 128:(tl + 1) * 128],
                            in_=xg[:, tl * IN + koff: tl * IN + koff + kw],
                            identity=id_sb[:],
                        )
                    xt = xtspool.tile([kw, TGW], BF16, name=f"xt{kc}",
                                      tag=f"xts{kc}")
                    nc.vector.tensor_copy(xt[0:kw, 0:gw], xtp[0:kw, 0:gw])
                    return xt

                last = ntl == 1
                nchunk_early = 3 if last else len(KCH)
                xts = [emit_chunk(kc) for kc in range(nchunk_early)]

                # --- gating (kc3 step deferred for the last group) ---
                glp = glppool.tile([D * E, TGW], F32, space="PSUM", tag="glp")
                for kc in range(nchunk_early):
                    nc.tensor.matmul(glp[:, 0:gw], wg_sb[kc][:],
                                     xts[kc][0:KCH[kc][1], 0:gw],
                                     start=(kc == 0),
                                     stop=(kc == len(KCH) - 1),
                                     skip_group_check=True)

                def finish_gating():
                    expsb = gsbpool.tile([D * E, TGW], F32, tag="expsb")
                    nc.scalar.activation(expsb[:, 0:gw], glp[:, 0:gw],
                                         mybir.ActivationFunctionType.Exp,
                                         bias=bg_sb[:, 0:1])
                    masked = gsbpool.tile([D * E, TGW], BF16, tag="masked")
                    nc.vector.tensor_tensor(
                        out=masked[:, 0:gw], in0=expsb[:, 0:gw],
                        in1=oh_sb[:, t0 * 128: t0 * 128 + gw],
                        op=mybir.AluOpType.mult)
                    gslp = glppool.tile([D * E, TGW], F32, space="PSUM",
                                        tag="glp")
                    nc.tensor.matmul(gslp[0:8, 0:gw], sel_sb[:],
                                     masked[:, 0:gw], start=True, stop=True)
                    gssb = gsbpool.tile([8, TGW], F32, tag="gssb")
                    nc.scalar.activation(gssb[:, 0:gw], gslp[0:8, 0:gw],
                                         mybir.ActivationFunctionType.Copy)
                    return masked, gssb

                if not last:
                    masked, gssb = finish_gating()

                # --- experts ---
                sp = sppool.tile([E, 2 * TGW], F32, space="PSUM", tag="spp")
                if ntl == 1:
                    # Last tiny group: accumulate each expert's k0..k2 h1
                    # partials in SBUF (DVE) so the two h1 PSUM banks recycle
                    # and all 8 experts' bulk work runs during the gather
                    # stream; only the kc3 step + epilogues trail it.
                    h1acc = []
                    for kc in range(3):
                        for e in range(E):
                            h1p = h1ppool.tile([H1, TGW], F32, space="PSUM",
                                               tag="h1p")
                            nc.tensor.matmul(
                                h1p[:, 0:gw],
                                w1_sb[kc][:, e * H1:(e + 1) * H1],
                                xts[kc][0:KCH[kc][1], 0:gw],
                                start=True, stop=True)
                            if kc == 0:
                                acc = cpool.tile([H1, 128], F32,
                                                 name=f"h1acc{e}",
                                                 tag=f"h1acc{e}")
                                h1acc.append(acc)
                                nc.vector.tensor_copy(acc[:, 0:gw],
                                                      h1p[:, 0:gw])
                            else:
                                nc.vector.tensor_tensor(
                                    out=h1acc[e][:, 0:gw],
                                    in0=h1acc[e][:, 0:gw], in1=h1p[:, 0:gw],
                                    op=mybir.AluOpType.add)
                    # now the stream-end-dependent work: kc3 transpose,
                    # gating kc3 + softmax, then the expert epilogues
                    xts.append(emit_chunk(3))
                    nc.tensor.matmul(glp[:, 0:gw], wg_sb[3][:],
                                     xts[3][0:KCH[3][1], 0:gw],
                                     start=False, stop=True,
                                     skip_group_check=True)
                    masked, gssb = finish_gating()
                    for e in range(E):
                        h1p = h1ppool.tile([H1, TGW], F32, space="PSUM",
                                           tag="h1p")
                        nc.tensor.matmul(
                            h1p[:, 0:gw], w1_sb[3][:, e * H1:(e + 1) * H1],
                            xts[3][0:KCH[3][1], 0:gw], start=True, stop=True)
                        h1f = h1spool.tile([H1, TGW], F32, tag="h1f")
                        nc.vector.tensor_tensor(
                            out=h1f[:, 0:gw], in0=h1acc[e][:, 0:gw],
                            in1=h1p[:, 0:gw], op=mybir.AluOpType.add)
                        h1s = h1spool.tile([H1, TGW], BF16, tag="h1s")
                        nc.scalar.activation(h1s[:, 0:gw], h1f[:, 0:gw],
                                             mybir.ActivationFunctionType.Relu,
                                             bias=b1_sb[:, e:e + 1])
                        h2p = h2ppool.tile([H2, TGW], F32, space="PSUM",
                                           tag="h2p")
                        nc.tensor.matmul(h2p[:, 0:gw],
                                         w2_sb[:, e * H2:(e + 1) * H2],
                                         h1s[:, 0:gw], start=True, stop=True)
                        h2a = h2spool.tile([H2, TGW], F32, tag="h2a")
                        nc.vector.tensor_tensor(
                            out=h2a[:, 0:gw], in0=h2p[:, 0:gw],
                            in1=b2_sb[:, e:e + 1].to_broadcast([H2, gw]),
                            op=mybir.AluOpType.add)
                        h2s = h2spool.tile([H2, TGW], BF16, tag="h2s")
                        nc.vector.tensor_scalar_max(h2s[:, 0:gw],
                                                    h2a[:, 0:gw], 0.0)
                        nc.tensor.matmul(sp[:, 0:gw],
                                         wo_sb[:, e * 8:(e + 1) * 8],
                                         h2s[:, 0:gw],
                                         start=(e == 0), stop=(e == E - 1),
                                         skip_group_check=True)
                else:
                    for e in range(E):
                        h1p = h1ppool.tile([H1, TGW], F32, space="PSUM",
                                           tag="h1p")
                        for kc in range(len(KCH)):
                            nc.tensor.matmul(
                                h1p[:, 0:gw], w1_sb[kc][:, e * H1:(e + 1) * H1],
                                xts[kc][0:KCH[kc][1], 0:gw],
                                start=(kc == 0), stop=(kc == len(KCH) - 1))
                        h1s = h1spool.tile([H1, TGW], BF16, tag="h1s")
                        nc.scalar.activation(h1s[:, 0:gw], h1p[:, 0:gw],
                                             mybir.ActivationFunctionType.Relu,
                                             bias=b1_sb[:, e:e + 1])
                        h2p = h2ppool.tile([H2, TGW], F32, space="PSUM",
                                           tag="h2p")
                        nc.tensor.matmul(h2p[:, 0:gw],
                                         w2_sb[:, e * H2:(e + 1) * H2],
                                         h1s[:, 0:gw], start=True, stop=True)
                        # h2 bias+relu on DVE (keeps ACT off the tail path)
                        h2a = h2spool.tile([H2, TGW], F32, tag="h2a")
                        nc.vector.tensor_tensor(
                            out=h2a[:, 0:gw], in0=h2p[:, 0:gw],
                            in1=b2_sb[:, e:e + 1].to_broadcast([H2, gw]),
                            op=mybir.AluOpType.add)
                        h2s = h2spool.tile([H2, TGW], BF16, tag="h2s")
                        nc.vector.tensor_scalar_max(h2s[:, 0:gw],
                                                    h2a[:, 0:gw], 0.0)
                        nc.tensor.matmul(sp[:, 0:gw],
                                         wo_sb[:, e * 8:(e + 1) * 8],
                                         h2s[:, 0:gw],
                                         start=(e == 0), stop=(e == E - 1),
                                         skip_group_check=True)

                # --- final: logits = (sum_e gsel*s)/denom + bo ---
                msb = finpool.tile([E, TGW], BF16, tag="msb")
                nc.vector.tensor_tensor(out=msb[:, 0:gw], in0=sp[:, 0:gw],
                                        in1=gssb[:, 0:gw],
                                        op=mybir.AluOpType.mult)
                updn = sppool.tile([E, 2 * TGW], F32, space="PSUM", tag="spp")
                nc.tensor.matmul(updn[0:1, 0:gw], on8_sb[:], msb[:, 0:gw],
                                 start=True, stop=True)
                nc.tensor.matmul(updn[0:1, TGW:TGW + gw], on32_sb[:],
                                 masked[:, 0:gw], start=True, stop=True)
                rr = finpool.tile([1, TGW], F32, tag="rr")
                nc.vector.reciprocal(rr[0:1, 0:gw], updn[0:1, TGW:TGW + gw])
                lsb = finpool.tile([1, TGW], F32, tag="lsb")
                nc.vector.tensor_tensor(out=lsb[0:1, 0:gw],
                                        in0=updn[0:1, 0:gw],
                                        in1=rr[0:1, 0:gw],
                                        op=mybir.AluOpType.mult)
                nc.vector.tensor_scalar_add(lsb[0:1, 0:gw], lsb[0:1, 0:gw],
                                            float(bo_val))
                nc.sync.dma_start(out=out[0:1, t0 * 128: t0 * 128 + gw],
                                  in_=lsb[0:1, 0:gw])

    nc.compile()
    return nc


def kernel(**inputs):
    features = np.asarray(inputs["features"])
    domain = np.asarray(inputs["domain_indicator"])
    emb = np.asarray(inputs["emb"], dtype=np.float32)
    W1 = np.asarray(inputs["W1"], dtype=np.float32)
    b1 = np.asarray(inputs["b1"], dtype=np.float32)
    W2 = np.asarray(inputs["W2"], dtype=np.float32)
    b2 = np.asarray(inputs["b2"], dtype=np.float32)
    Wg = np.asarray(inputs["Wg"], dtype=np.float32)
    bg = np.asarray(inputs["bg"], dtype=np.float32)
    Wo = np.asarray(inputs["Wo"], dtype=np.float32)
    bo = np.asarray(inputs["bo"], dtype=np.float32)

    bo_val = float(bo.reshape(-1)[0])
    key = ("m3oe", bo_val)
    if key not in _cache:
        _cache[key] = _build(bo_val)
    nc = _cache[key]

    # ---- host-side prep (shared across cores) ----
    emb_flat = np.ascontiguousarray(
        emb.reshape(F * V, DK)).astype(ml_dtypes.bfloat16)
    # flattened per-(sample,field) row index into emb_flat
    idx_all = (features.astype(np.int64) +
               (np.arange(F, dtype=np.int64) * V)[None, :]).astype(np.int32)

    w1k = []
    wgk = []
    for koff, kw in KCH:
        # w1k[i, e*H1+h] = W1[e, koff+i, h]
        w1k.append(np.ascontiguousarray(
            W1[:, koff:koff + kw, :].transpose(1, 0, 2).reshape(kw, E * H1)
        ).astype(ml_dtypes.bfloat16))
        wgk.append(np.ascontiguousarray(
            Wg[:, koff:koff + kw, :].transpose(1, 0, 2).reshape(kw, D * E)
        ).astype(ml_dtypes.bfloat16))
    w2c = np.ascontiguousarray(
        W2.transpose(1, 0, 2).reshape(H1, E * H2)).astype(ml_dtypes.bfloat16)
    # wo8[k, e*8+m] = Wo[k] iff m == e (so s_e lands on psum partition e)
    wo8 = np.zeros((H2, E * 8), dtype=ml_dtypes.bfloat16)
    wov = Wo.reshape(H2)
    for e in range(E):
        wo8[:, e * 8 + e] = wov
    sel8 = np.zeros((D * E, 8), dtype=ml_dtypes.bfloat16)
    for d in range(D):
        for e in range(E):
            sel8[d * 8 + e, e] = 1.0
    ones8 = np.ones((E, 1), dtype=ml_dtypes.bfloat16)
    ones32 = np.ones((D * E, 1), dtype=ml_dtypes.bfloat16)
    b1t = np.ascontiguousarray(b1.T)  # [H1, E]
    b2t = np.ascontiguousarray(b2.T)  # [H2, E]
    bgc = bg.reshape(D * E, 1).astype(np.float32)
    id128 = np.eye(128, dtype=ml_dtypes.bfloat16)

    shared = {
        "emb_flat": emb_flat, "w2c": w2c, "wo8": wo8, "sel8": sel8,
        "ones8": ones8, "ones32": ones32, "b1t": b1t, "b2t": b2t,
        "bgc": bgc, "id128": id128,
    }
    for i in range(len(KCH)):
        shared[f"w1k{i}"] = w1k[i]
        shared[f"wgk{i}"] = wgk[i]

    derep = np.repeat(np.arange(D), E)  # [32] domain of each (d,e) row
    in_maps = []
    for c in range(N_CORES):
        sl = slice(c * PC, (c + 1) * PC)
        # idx_core[p, t*F+f] = idx_all[c*PC + t*128 + p, f]
        idx_core = np.ascontiguousarray(
            idx_all[sl].reshape(NT, 128, F).transpose(1, 0, 2)
            .reshape(128, NT * F))
        dom = domain[sl].astype(np.int64)
        oh_core = (dom[None, :] == derep[:, None]).astype(np.float32)
        m = dict(shared)
        m["idx"] = idx_core
        m["oh"] = oh_core
        in_maps.append(m)

    global LAST_RESULT
    res = run_bass_kernel_spmd(nc, in_maps, core_ids=list(range(N_CORES)),
                               trace=TRACE, tmpdir=TRACE_TMPDIR)
    LAST_RESULT = res
    outs = [res.results[c]["out"].reshape(PC) for c in range(N_CORES)]
    return np.concatenate(outs).astype(np.float32)



# revision 25
# speedup vs baseline: 1.0037x; 1.0037x over previous
"""Trainium2 kernel for nn_M3oE: multi-domain MoE over 26 categorical embeddings.

Sharding: data-parallel over batch across 8 NeuronCores (2048 rows each),
embedding tables replicated in DRAM (gathered directly from HBM via
indirect DMA with inline f32->bf16 cast; tables never enter SBUF).

The kernel is gather-stream-bound: the SWDGE indirect DMA processes exactly
one offset per partition per instruction (HW-probed; multi-offset forms are
UB on the built-in Q7 ucode, and the custom gather ucode library is not
shipped in this image), so the 53248 embedding-row fetches per core cost
416 instructions at ~1.4us each (~1.09us Q7 ucode + ~0.31us issue gap)
= ~582us, regardless of batching. All compute hides under that stream; the
span beyond it is just NEFF startup (~10us) and the last group's compute
tail, which variable-size tile groups (4,4,4,3,1) keep small.

Per-core pipeline (matmuls in bf16, PSUM f32):
  1. indirect-DMA gather of 26 rows/sample into xg [128, tiles*416] bf16
  2. PE transpose of x chunks -> xT [416(k-chunks), cols] (DVE evict, bf16)
  3. 8 experts: h1T = relu(W1^T xT + b1) (ACT evict fuses bias+relu+cast);
     h2T = relu(W2^T h1T + b2) (DVE add+max, off ACT's critical path);
     s_e = Wo . h2T accumulated into one [8, cols] PSUM tile via
     zero-padded Wo blocks
  4. domain-gated softmax: glog = Wg^T xT ; exp via ACT (+bg bias);
     one-hot domain mask (host-built) picks the sample's domain; expert-sum
     matmul (sel8) compresses to [8, cols]; denominator via ones32 matmul
  5. logits = (sum_e gsel_e * s_e) / denom + bo   (division via DVE recip)
"""

import ml_dtypes
import numpy as np

import concourse.bacc as bacc
import concourse.mybir as mybir
import concourse.tile as tile
from concourse.bass import IndirectOffsetOnAxis
from concourse.bass_utils import run_bass_kernel_spmd

F = 26
V = 100000
DK = 16
D = 4
E = 8
H1 = 128
H2 = 64
B = 16384
IN = F * DK  # 416
N_CORES = 8
PC = B // N_CORES  # 2048 rows per core
NT = PC // 128  # 16 batch-tiles of 128
NTG = 4  # tile groups (legacy; see GROUPS)
TGW = 512  # max columns per tile group (one PSUM bank of f32)
TPG = NT // NTG  # legacy
# Variable-size tile groups: big groups while the gather stream dominates,
# small trailing groups to shrink the post-stream compute tail.
GROUPS = [(0, 4), (4, 4), (8, 4), (12, 2), (14, 1), (15, 1)]
KCH = [(0, 128), (128, 128), (256, 128), (384, 32)]  # k-chunks of IN=416

F32 = mybir.dt.float32
F32R = mybir.dt.float32r
BF16 = mybir.dt.bfloat16
I32 = mybir.dt.int32

_cache = {}

# test-harness knobs (unused when the harness calls kernel() directly)
TRACE = False
TRACE_TMPDIR = None
LAST_RESULT = None


def _build(bo_val: float):
    nc = bacc.Bacc("TRN2", target_bir_lowering=False, debug=False,
                   num_devices=N_CORES)

    emb = nc.dram_tensor("emb_flat", [F * V, DK], BF16, kind="ExternalInput")
    idx = nc.dram_tensor("idx", [128, NT * F], I32, kind="ExternalInput")
    w1k = [nc.dram_tensor(f"w1k{i}", [w, E * H1], BF16, kind="ExternalInput")
           for i, (_, w) in enumerate(KCH)]
    w2c = nc.dram_tensor("w2c", [H1, E * H2], BF16, kind="ExternalInput")
    wo8 = nc.dram_tensor("wo8", [H2, E * 8], BF16, kind="ExternalInput")
    wgk = [nc.dram_tensor(f"wgk{i}", [w, D * E], BF16, kind="ExternalInput")
           for i, (_, w) in enumerate(KCH)]
    sel8 = nc.dram_tensor("sel8", [D * E, 8], BF16, kind="ExternalInput")
    ones8 = nc.dram_tensor("ones8", [E, 1], BF16, kind="ExternalInput")
    ones32 = nc.dram_tensor("ones32", [D * E, 1], BF16, kind="ExternalInput")
    b1t = nc.dram_tensor("b1t", [H1, E], F32, kind="ExternalInput")
    b2t = nc.dram_tensor("b2t", [H2, E], F32, kind="ExternalInput")
    bgc = nc.dram_tensor("bgc", [D * E, 1], F32, kind="ExternalInput")
    oh = nc.dram_tensor("oh", [D * E, PC], F32, kind="ExternalInput")
    id128 = nc.dram_tensor("id128", [128, 128], BF16, kind="ExternalInput")
    out = nc.dram_tensor("out", [1, PC], F32, kind="ExternalOutput")

    with tile.TileContext(nc) as tc:
        with (
            tc.tile_pool(name="const", bufs=1) as cpool,
            tc.tile_pool(name="xg", bufs=4) as xgpool,
            tc.tile_pool(name="xts", bufs=2 * len(KCH)) as xtspool,
            tc.tile_pool(name="h1s", bufs=3) as h1spool,
            tc.tile_pool(name="h2s", bufs=2) as h2spool,
            tc.tile_pool(name="gsb", bufs=2) as gsbpool,
            tc.tile_pool(name="fin", bufs=2) as finpool,
            tc.tile_pool(name="xtp", bufs=2, space="PSUM") as xtppool,
            tc.tile_pool(name="h1p", bufs=2, space="PSUM") as h1ppool,
            tc.tile_pool(name="h2p", bufs=1, space="PSUM") as h2ppool,
            tc.tile_pool(name="glp", bufs=1, space="PSUM") as glppool,
            tc.tile_pool(name="spp", bufs=1, space="PSUM") as sppool,
        ):
            # --- load constants ---
            def cload(dram, shape, dtype=None):
                t = cpool.tile(shape, dtype or dram.dtype, tag=dram.name)
                nc.sync.dma_start(out=t[:], in_=dram[:])
                return t

            idx_sb = cload(idx, [128, NT * F], I32)
            w1_sb = [cload(w1k[i], [w, E * H1]) for i, (_, w) in enumerate(KCH)]
            w2_sb = cload(w2c, [H1, E * H2])
            wo_sb = cload(wo8, [H2, E * 8])
            wg_sb = [cload(wgk[i], [w, D * E]) for i, (_, w) in enumerate(KCH)]
            sel_sb = cload(sel8, [D * E, 8])
            on8_sb = cload(ones8, [E, 1])
            on32_sb = cload(ones32, [D * E, 1])
            b1_sb = cload(b1t, [H1, E])
            b2_sb = cload(b2t, [H2, E])
            bg_sb = cload(bgc, [D * E, 1])
            oh_sb = cload(oh, [D * E, PC])
            id_sb = cload(id128, [128, 128])

            for t0, ntl in GROUPS:
                gw = ntl * 128  # columns in this group
                # --- gather: this group's embedding rows ---
                # SWDGE contract (HW-probed): one offset per partition per
                # instruction, offset read from the offset AP's first column
                # at each partition. F instructions per batch-tile is the
                # floor (~1.4us each: ~1.09us Q7 ucode + ~0.31us issue gap).
                xg = xgpool.tile([128, TPG * IN], BF16, tag="xg")
                for j in range(ntl * F):
                    jj = t0 * F + j
                    nc.gpsimd.indirect_dma_start(
                        out=xg[:, j * DK:(j + 1) * DK],
                        out_offset=None,
                        in_=emb[:],
                        in_offset=IndirectOffsetOnAxis(
                            ap=idx_sb[:, jj:jj + 1], axis=0),
                    )

                # --- transpose x -> xT per k-chunk (PE), evict to SBUF ---
                # For the last 1-tile group, defer the kc3 transpose (which
                # waits on the very last gather) so it doesn't head-of-line
                # block the kc0-2 expert matmuls on the in-order PE queue.
                def emit_chunk(kc):
                    koff, kw = KCH[kc]
                    xtp = xtppool.tile([128, TGW], BF16, space="PSUM",
                                       name=f"xtp{kc}", tag="xtp")
                    for tl in range(ntl):
                        nc.tensor.transpose(
                            out=xtp[0:kw, tl * 128:(tl + 1) * 128],
                            in_=xg[:, tl * IN + koff: tl * IN + koff + kw],
                            identity=id_sb[:],
                        )
                    xt = xtspool.tile([kw, TGW], BF16, name=f"xt{kc}",
                                      tag=f"xts{kc}")
                    nc.vector.tensor_copy(xt[0:kw, 0:gw], xtp[0:kw, 0:gw])
                    return xt

                last = ntl == 1
                nchunk_early = 3 if last else len(KCH)
                xts = [emit_chunk(kc) for kc in range(nchunk_early)]

                # --- gating (kc3 step deferred for the last group) ---
                glp = glppool.tile([D * E, TGW], F32, space="PSUM", tag="glp")
                for kc in range(nchunk_early):
                    nc.tensor.matmul(glp[:, 0:gw], wg_sb[kc][:],
                                     xts[kc][0:KCH[kc][1], 0:gw],
                                     start=(kc == 0),
                                     stop=(kc == len(KCH) - 1),
                                     skip_group_check=True)

                def finish_gating():
                    expsb = gsbpool.tile([D * E, TGW], F32, tag="expsb")
                    nc.scalar.activation(expsb[:, 0:gw], glp[:, 0:gw],
                                         mybir.ActivationFunctionType.Exp,
                                         bias=bg_sb[:, 0:1])
                    masked = gsbpool.tile([D * E, TGW], BF16, tag="masked")
                    nc.vector.tensor_tensor(
                        out=masked[:, 0:gw], in0=expsb[:, 0:gw],
                        in1=oh_sb[:, t0 * 128: t0 * 128 + gw],
                        op=mybir.AluOpType.mult)
                    gslp = glppool.tile([D * E, TGW], F32, space="PSUM",
                                        tag="glp")
                    nc.tensor.matmul(gslp[0:8, 0:gw], sel_sb[:],
                                     masked[:, 0:gw], start=True, stop=True)
                    gssb = gsbpool.tile([8, TGW], F32, tag="gssb")
                    nc.scalar.activation(gssb[:, 0:gw], gslp[0:8, 0:gw],
                                         mybir.ActivationFunctionType.Copy)
                    return masked, gssb

                if not last:
                    masked, gssb = finish_gating()

                # --- experts ---
                sp = sppool.tile([E, 2 * TGW], F32, space="PSUM", tag="spp")
                if ntl == 1:
                    # Last tiny group: accumulate each expert's k0..k2 h1
                    # partials in SBUF (DVE) so the two h1 PSUM banks recycle
                    # and all 8 experts' bulk work runs during the gather
                    # stream; only the kc3 step + epilogues trail it.
                    h1acc = []
                    for kc in range(3):
                        for e in range(E):
                            h1p = h1ppool.tile([H1, TGW], F32, space="PSUM",
                                               tag="h1p")
                            nc.tensor.matmul(
                                h1p[:, 0:gw],
                                w1_sb[kc][:, e * H1:(e + 1) * H1],
                                xts[kc][0:KCH[kc][1], 0:gw],
                                start=True, stop=True)
                            if kc == 0:
                                acc = cpool.tile([H1, 128], F32,
                                                 name=f"h1acc{e}",
                                                 tag=f"h1acc{e}")
                                h1acc.append(acc)
                                nc.vector.tensor_copy(acc[:, 0:gw],
                                                      h1p[:, 0:gw])
                            else:
                                nc.vector.tensor_tensor(
                                    out=h1acc[e][:, 0:gw],
                                    in0=h1acc[e][:, 0:gw], in1=h1p[:, 0:gw],
                                    op=mybir.AluOpType.add)
                    # now the stream-end-dependent work: kc3 transpose,
                    # gating kc3 + softmax, then the expert epilogues
                    xts.append(emit_chunk(3))
                    nc.tensor.matmul(glp[:, 0:gw], wg_sb[3][:],
                                     xts[3][0:KCH[3][1], 0:gw],
                                     start=False, stop=True,
                                     skip_group_check=True)
                    masked, gssb = finish_gating()
                    for e in range(E):
                        h1p = h1ppool.tile([H1, TGW], F32, space="PSUM",
                                           tag="h1p")
                        nc.tensor.matmul(
                            h1p[:, 0:gw], w1_sb[3][:, e * H1:(e + 1) * H1],
                            xts[3][0:KCH[3][1], 0:gw], start=True, stop=True)
                        h1f = h1spool.tile([H1, TGW], F32, tag="h1f")
                        nc.vector.tensor_tensor(
                            out=h1f[:, 0:gw], in0=h1acc[e][:, 0:gw],
                            in1=h1p[:, 0:gw], op=mybir.AluOpType.add)
                        h1s = h1spool.tile([H1, TGW], BF16, tag="h1s")
                        nc.scalar.activation(h1s[:, 0:gw], h1f[:, 0:gw],
                                             mybir.ActivationFunctionType.Relu,
                                             bias=b1_sb[:, e:e + 1])
                        h2p = h2ppool.tile([H2, TGW], F32, space="PSUM",
                                           tag="h2p")
                        nc.tensor.matmul(h2p[:, 0:gw],
                                         w2_sb[:, e * H2:(e + 1) * H2],
                                         h1s[:, 0:gw], start=True, stop=True)
                        h2a = h2spool.tile([H2, TGW], F32, tag="h2a")
                        nc.vector.tensor_tensor(
                            out=h2a[:, 0:gw], in0=h2p[:, 0:gw],
                            in1=b2_sb[:, e:e + 1].to_broadcast([H2, gw]),
                            op=mybir.AluOpType.add)
                        h2s = h2spool.tile([H2, TGW], BF16, tag="h2s")
                        nc.vector.tensor_scalar_max(h2s[:, 0:gw],
                                                    h2a[:, 0:gw], 0.0)
                        nc.tensor.matmul(sp[:, 0:gw],
                                         wo_sb[:, e * 8:(e + 1) * 8],
                                         h2s[:, 0:gw],
                                         start=(e == 0), stop=(e == E - 1),
                                         skip_group_check=True)
                else:
                    for e in range(E):
                        h1p = h1ppool.tile([H1, TGW], F32, space="PSUM",
                                           tag="h1p")
                        for kc in range(len(KCH)):
                            nc.tensor.matmul(
                                h1p[:, 0:gw], w1_sb[kc][:, e * H1:(e + 1) * H1],
                                xts[kc][0:KCH[kc][1], 0:gw],
                                start=(kc == 0), stop=(kc == len(KCH) - 1))
                        h1s = h1spool.tile([H1, TGW], BF16, tag="h1s")
                        nc.scalar.activation(h1s[:, 0:gw], h1p[:, 0:gw],
                                             mybir.ActivationFunctionType.Relu,
                                             bias=b1_sb[:, e:e + 1])
                        h2p = h2ppool.tile([H2, TGW], F32, space="PSUM",
                                           tag="h2p")
                        nc.tensor.matmul(h2p[:, 0:gw],
                                         w2_sb[:, e * H2:(e + 1) * H2],
                                         h1s[:, 0:gw], start=True, stop=True)
                        # h2 bias+relu on DVE (keeps ACT off the tail path)
                        h2a = h2spool.tile([H2, TGW], F32, tag="h2a")
                        nc.vector.tensor_tensor(
                            out=h2a[:, 0:gw], in0=h2p[:, 0:gw],
                            in1=b2_sb[:, e:e + 1].to_broadcast([H2, gw]),
                            op=mybir.AluOpType.add)
                        h2s = h2spool.tile([H2, TGW], BF16, tag="h2s")
                        nc.vector.tensor_scalar_max(h2s[:, 0:gw],
                                                    h2a[:, 0:gw], 0.0)
                        nc.tensor.matmul(sp[:, 0:gw],
                                         wo_sb[:, e * 8:(e + 1) * 8],
                                         h2s[:, 0:gw],
                                         start=(e == 0), stop=(e == E - 1),
                                         skip_group_check=True)

                # --- final: logits = (sum_e gsel*s)/denom + bo ---
                msb = finpool.tile([E, TGW], BF16, tag="msb")
                nc.vector.tensor_tensor(out=msb[:, 0:gw], in0=sp[:, 0:gw],
                                        in1=gssb[:, 0:gw],
                                        op=mybir.AluOpType.mult)
                updn = sppool.tile([E, 2 * TGW], F32, space="PSUM", tag="spp")
                nc.tensor.matmul(updn[0:1, 0:gw], on8_sb[:], msb[:, 0:gw],
                                 start=True, stop=True)
                nc.tensor.matmul(updn[0:1, TGW:TGW + gw], on32_sb[:],
                                 masked[:, 0:gw], start=True, stop=True)
                rr = finpool.tile([1, TGW], F32, tag="rr")
                nc.vector.reciprocal(rr[0:1, 0:gw], updn[0:1, TGW:TGW + gw])
                lsb = finpool.tile([1, TGW], F32, tag="lsb")
                nc.vector.tensor_tensor(out=lsb[0:1, 0:gw],
                                        in0=updn[0:1, 0:gw],
                                        in1=rr[0:1, 0:gw],
                                        op=mybir.AluOpType.mult)
                nc.vector.tensor_scalar_add(lsb[0:1, 0:gw], lsb[0:1, 0:gw],
                                            float(bo_val))
                nc.sync.dma_start(out=out[0:1, t0 * 128: t0 * 128 + gw],
                                  in_=lsb[0:1, 0:gw])

    nc.compile()
    return nc


def kernel(**inputs):
    features = np.asarray(inputs["features"])
    domain = np.asarray(inputs["domain_indicator"])
    emb = np.asarray(inputs["emb"], dtype=np.float32)
    W1 = np.asarray(inputs["W1"], dtype=np.float32)
    b1 = np.asarray(inputs["b1"], dtype=np.float32)
    W2 = np.asarray(inputs["W2"], dtype=np.float32)
    b2 = np.asarray(inputs["b2"], dtype=np.float32)
    Wg = np.asarray(inputs["Wg"], dtype=np.float32)
    bg = np.asarray(inputs["bg"], dtype=np.float32)
    Wo = np.asarray(inputs["Wo"], dtype=np.float32)
    bo = np.asarray(inputs["bo"], dtype=np.float32)

    bo_val = float(bo.reshape(-1)[0])
    key = ("m3oe", bo_val)
    if key not in _cache:
        _cache[key] = _build(bo_val)
    nc = _cache[key]

    # ---- host-side prep (shared across cores) ----
    emb_flat = np.ascontiguousarray(
        emb.reshape(F * V, DK)).astype(ml_dtypes.bfloat16)
    # flattened per-(sample,field) row index into emb_flat
    idx_all = (features.astype(np.int64) +
               (np.arange(F, dtype=np.int64) * V)[None, :]).astype(np.int32)

    w1k = []
    wgk = []
    for koff, kw in KCH:
        # w1k[i, e*H1+h] = W1[e, koff+i, h]
        w1k.append(np.ascontiguousarray(
            W1[:, koff:koff + kw, :].transpose(1, 0, 2).reshape(kw, E * H1)
        ).astype(ml_dtypes.bfloat16))
        wgk.append(np.ascontiguousarray(
            Wg[:, koff:koff + kw, :].transpose(1, 0, 2).reshape(kw, D * E)
        ).astype(ml_dtypes.bfloat16))
    w2c = np.ascontiguousarray(
        W2.transpose(1, 0, 2).reshape(H1, E * H2)).astype(ml_dtypes.bfloat16)
    # wo8[k, e*8+m] = Wo[k] iff m == e (so s_e lands on psum partition e)
    wo8 = np.zeros((H2, E * 8), dtype=ml_dtypes.bfloat16)
    wov = Wo.reshape(H2)
    for e in range(E):
        wo8[:, e * 8 + e] = wov
    sel8 = np.zeros((D * E, 8), dtype=ml_dtypes.bfloat16)
    for d in range(D):
        for e in range(E):
            sel8[d * 8 + e, e] = 1.0
    ones8 = np.ones((E, 1), dtype=ml_dtypes.bfloat16)
    ones32 = np.ones((D * E, 1), dtype=ml_dtypes.bfloat16)
    b1t = np.ascontiguousarray(b1.T)  # [H1, E]
    b2t = np.ascontiguousarray(b2.T)  # [H2, E]
    bgc = bg.reshape(D * E, 1).astype(np.float32)
    id128 = np.eye(128, dtype=ml_dtypes.bfloat16)

    shared = {
        "emb_flat": emb_flat, "w2c": w2c, "wo8": wo8, "sel8": sel8,
        "ones8": ones8, "ones32": ones32, "b1t": b1t, "b2t": b2t,
        "bgc": bgc, "id128": id128,
    }
    for i in range(len(KCH)):
        shared[f"w1k{i}"] = w1k[i]
        shared[f"wgk{i}"] = wgk[i]

    derep = np.repeat(np.arange(D), E)  # [32] domain of each (d,e) row
    in_maps = []
    for c in range(N_CORES):
        sl = slice(c * PC, (c + 1) * PC)
        # idx_core[p, t*F+f] = idx_all[c*PC + t*128 + p, f]
        idx_core = np.ascontiguousarray(
            idx_all[sl].reshape(NT, 128, F).transpose(1, 0, 2)
            .reshape(128, NT * F))
        dom = domain[sl].astype(np.int64)
        oh_core = (dom[None, :] == derep[:, None]).astype(np.float32)
        m = dict(shared)
        m["idx"] = idx_core
        m["oh"] = oh_core
        in_maps.append(m)

    global LAST_RESULT
    res = run_bass_kernel_spmd(nc, in_maps, core_ids=list(range(N_CORES)),
                               trace=TRACE, tmpdir=TRACE_TMPDIR)
    LAST_RESULT = res
    outs = [res.results[c]["out"].reshape(PC) for c in range(N_CORES)]
    return np.concatenate(outs).astype(np.float32)



# revision 27
# speedup vs baseline: 1.0074x; 1.0036x over previous
"""Trainium2 kernel for nn_M3oE: multi-domain MoE over 26 categorical embeddings.

Sharding: data-parallel over batch across 8 NeuronCores (2048 rows each),
embedding tables replicated in DRAM (gathered directly from HBM via
indirect DMA with inline f32->bf16 cast; tables never enter SBUF).

The kernel is gather-stream-bound: the SWDGE indirect DMA processes exactly
one offset per partition per instruction (HW-probed; multi-offset forms are
UB on the built-in Q7 ucode, and the custom gather ucode library is not
shipped in this image), so the 53248 embedding-row fetches per core cost
416 instructions at ~1.4us each (~1.09us Q7 ucode + ~0.31us issue gap)
= ~582us, regardless of batching. All compute hides under that stream; the
span beyond it is just NEFF startup (~10us) and the trailing compute,
which variable-size tile groups (4,4,4,2,1,1) keep small: the two 1-tile
tail groups defer their kc3 transpose + gating step and accumulate h1
k0..k2 partials in SBUF via DVE, so their bulk PE work overlaps the end
of the gather stream instead of queueing behind it.

Per-core pipeline (matmuls in bf16, PSUM f32):
  1. indirect-DMA gather of 26 rows/sample into xg [128, tiles*416] bf16
  2. PE transpose of x chunks -> xT [416(k-chunks), cols] (DVE evict, bf16)
  3. 8 experts: h1T = relu(W1^T xT + b1) (ACT evict fuses bias+relu+cast);
     h2T = relu(W2^T h1T + b2) (DVE add+max, off ACT's critical path);
     s_e = Wo . h2T accumulated into one [8, cols] PSUM tile via
     zero-padded Wo blocks
  4. domain-gated softmax: glog = Wg^T xT ; exp via ACT (+bg bias);
     one-hot domain mask (host-built) picks the sample's domain; expert-sum
     matmul (sel8) compresses to [8, cols]; denominator via ones32 matmul
  5. logits = (sum_e gsel_e * s_e) / denom + bo   (division via DVE recip)
"""

import ml_dtypes
import numpy as np

import concourse.bacc as bacc
import concourse.mybir as mybir
import concourse.tile as tile
from concourse.bass import IndirectOffsetOnAxis
from concourse.bass_utils import run_bass_kernel_spmd

F = 26
V = 100000
DK = 16
D = 4
E = 8
H1 = 128
H2 = 64
B = 16384
IN = F * DK  # 416
N_CORES = 8
PC = B // N_CORES  # 2048 rows per core
NT = PC // 128  # 16 batch-tiles of 128
NTG = 4  # tile groups (legacy; see GROUPS)
TGW = 512  # max columns per tile group (one PSUM bank of f32)
TPG = NT // NTG  # legacy
# Variable-size tile groups: big groups while the gather stream dominates,
# small trailing groups to shrink the post-stream compute tail.
GROUPS = [(0, 4), (4, 4), (8, 4), (12, 2), (14, 1), (15, 1)]
KCH = [(0, 128), (128, 128), (256, 128), (384, 32)]  # k-chunks of IN=416

F32 = mybir.dt.float32
F32R = mybir.dt.float32r
BF16 = mybir.dt.bfloat16
I32 = mybir.dt.int32

_cache = {}

# test-harness knobs (unused when the harness calls kernel() directly)
TRACE = False
TRACE_TMPDIR = None
LAST_RESULT = None


def _build(bo_val: float, use_b2: bool):
    nc = bacc.Bacc("TRN2", target_bir_lowering=False, debug=False,
                   num_devices=N_CORES)

    emb = nc.dram_tensor("emb_flat", [F * V, DK], BF16, kind="ExternalInput")
    idx = nc.dram_tensor("idx", [128, NT * F], I32, kind="ExternalInput")
    w1k = [nc.dram_tensor(f"w1k{i}", [w, E * H1], BF16, kind="ExternalInput")
           for i, (_, w) in enumerate(KCH)]
    w2c = nc.dram_tensor("w2c", [H1, E * H2], BF16, kind="ExternalInput")
    wo8 = nc.dram_tensor("wo8", [H2, E * 8], BF16, kind="ExternalInput")
    wgk = [nc.dram_tensor(f"wgk{i}", [w, D * E], BF16, kind="ExternalInput")
           for i, (_, w) in enumerate(KCH)]
    sel8 = nc.dram_tensor("sel8", [D * E, 8], BF16, kind="ExternalInput")
    ones8 = nc.dram_tensor("ones8", [E, 1], BF16, kind="ExternalInput")
    ones32 = nc.dram_tensor("ones32", [D * E, 1], BF16, kind="ExternalInput")
    b1t = nc.dram_tensor("b1t", [H1, E], F32, kind="ExternalInput")
    b2t = nc.dram_tensor("b2t", [H2, E], F32, kind="ExternalInput")
    bgc = nc.dram_tensor("bgc", [D * E, 1], F32, kind="ExternalInput")
    oh = nc.dram_tensor("oh", [D * E, PC], F32, kind="ExternalInput")
    id128 = nc.dram_tensor("id128", [128, 128], BF16, kind="ExternalInput")
    out = nc.dram_tensor("out", [1, PC], F32, kind="ExternalOutput")

    with tile.TileContext(nc) as tc:
        with (
            tc.tile_pool(name="const", bufs=1) as cpool,
            tc.tile_pool(name="xg", bufs=4) as xgpool,
            tc.tile_pool(name="xts", bufs=2 * len(KCH)) as xtspool,
            tc.tile_pool(name="h1s", bufs=3) as h1spool,
            tc.tile_pool(name="h2s", bufs=2) as h2spool,
            tc.tile_pool(name="gsb", bufs=2) as gsbpool,
            tc.tile_pool(name="fin", bufs=2) as finpool,
            tc.tile_pool(name="xtp", bufs=2, space="PSUM") as xtppool,
            tc.tile_pool(name="h1p", bufs=2, space="PSUM") as h1ppool,
            tc.tile_pool(name="h2p", bufs=1, space="PSUM") as h2ppool,
            tc.tile_pool(name="glp", bufs=1, space="PSUM") as glppool,
            tc.tile_pool(name="spp", bufs=1, space="PSUM") as sppool,
        ):
            # --- load constants ---
            def cload(dram, shape, dtype=None):
                t = cpool.tile(shape, dtype or dram.dtype, tag=dram.name)
                nc.sync.dma_start(out=t[:], in_=dram[:])
                return t

            idx_sb = cload(idx, [128, NT * F], I32)
            w1_sb = [cload(w1k[i], [w, E * H1]) for i, (_, w) in enumerate(KCH)]
            w2_sb = cload(w2c, [H1, E * H2])
            wo_sb = cload(wo8, [H2, E * 8])
            wg_sb = [cload(wgk[i], [w, D * E]) for i, (_, w) in enumerate(KCH)]
            sel_sb = cload(sel8, [D * E, 8])
            on8_sb = cload(ones8, [E, 1])
            on32_sb = cload(ones32, [D * E, 1])
            b1_sb = cload(b1t, [H1, E])
            b2_sb = cload(b2t, [H2, E])
            bg_sb = cload(bgc, [D * E, 1])
            oh_sb = cload(oh, [D * E, PC])
            id_sb = cload(id128, [128, 128])

            for t0, ntl in GROUPS:
                gw = ntl * 128  # columns in this group
                # --- gather: this group's embedding rows ---
                # SWDGE contract (HW-probed): one offset per partition per
                # instruction, offset read from the offset AP's first column
                # at each partition. F instructions per batch-tile is the
                # floor (~1.4us each: ~1.09us Q7 ucode + ~0.31us issue gap).
                xg = xgpool.tile([128, TPG * IN], BF16, tag="xg")
                for j in range(ntl * F):
                    jj = t0 * F + j
                    nc.gpsimd.indirect_dma_start(
                        out=xg[:, j * DK:(j + 1) * DK],
                        out_offset=None,
                        in_=emb[:],
                        in_offset=IndirectOffsetOnAxis(
                            ap=idx_sb[:, jj:jj + 1], axis=0),
                    )

                # --- transpose x -> xT per k-chunk (PE), evict to SBUF ---
                # For the last 1-tile group, defer the kc3 transpose (which
                # waits on the very last gather) so it doesn't head-of-line
                # block the kc0-2 expert matmuls on the in-order PE queue.
                def emit_chunk(kc):
                    koff, kw = KCH[kc]
                    xtp = xtppool.tile([128, TGW], BF16, space="PSUM",
                                       name=f"xtp{kc}", tag="xtp")
                    for tl in range(ntl):
                        nc.tensor.transpose(
                            out=xtp[0:kw, tl * 128:(tl + 1) * 128],
                            in_=xg[:, tl * IN + koff: tl * IN + koff + kw],
                            identity=id_sb[:],
                        )
                    xt = xtspool.tile([kw, TGW], BF16, name=f"xt{kc}",
                                      tag=f"xts{kc}")
                    nc.vector.tensor_copy(xt[0:kw, 0:gw], xtp[0:kw, 0:gw])
                    return xt

                last = ntl == 1
                nchunk_early = 3 if last else len(KCH)
                xts = [emit_chunk(kc) for kc in range(nchunk_early)]

                # --- gating (kc3 step deferred for the last group) ---
                glp = glppool.tile([D * E, TGW], F32, space="PSUM", tag="glp")
                for kc in range(nchunk_early):
                    nc.tensor.matmul(glp[:, 0:gw], wg_sb[kc][:],
                                     xts[kc][0:KCH[kc][1], 0:gw],
                                     start=(kc == 0),
                                     stop=(kc == len(KCH) - 1),
                                     skip_group_check=True)

                def finish_gating():
                    expsb = gsbpool.tile([D * E, TGW], F32, tag="expsb")
                    nc.scalar.activation(expsb[:, 0:gw], glp[:, 0:gw],
                                         mybir.ActivationFunctionType.Exp,
                                         bias=bg_sb[:, 0:1])
                    masked = gsbpool.tile([D * E, TGW], BF16, tag="masked")
                    nc.vector.tensor_tensor(
                        out=masked[:, 0:gw], in0=expsb[:, 0:gw],
                        in1=oh_sb[:, t0 * 128: t0 * 128 + gw],
                        op=mybir.AluOpType.mult)
                    gslp = glppool.tile([D * E, TGW], F32, space="PSUM",
                                        tag="glp")
                    nc.tensor.matmul(gslp[0:8, 0:gw], sel_sb[:],
                                     masked[:, 0:gw], start=True, stop=True)
                    gssb = gsbpool.tile([8, TGW], F32, tag="gssb")
                    nc.scalar.activation(gssb[:, 0:gw], gslp[0:8, 0:gw],
                                         mybir.ActivationFunctionType.Copy)
                    return masked, gssb

                if not last:
                    masked, gssb = finish_gating()

                # --- experts ---
                sp = sppool.tile([E, 2 * TGW], F32, space="PSUM", tag="spp")
                if ntl == 1:
                    # Last tiny group: accumulate each expert's k0..k2 h1
                    # partials in SBUF (DVE) so the two h1 PSUM banks recycle
                    # and all 8 experts' bulk work runs during the gather
                    # stream; only the kc3 step + epilogues trail it.
                    h1acc = []
                    for kc in range(3):
                        for e in range(E):
                            h1p = h1ppool.tile([H1, TGW], F32, space="PSUM",
                                               tag="h1p")
                            nc.tensor.matmul(
                                h1p[:, 0:gw],
                                w1_sb[kc][:, e * H1:(e + 1) * H1],
                                xts[kc][0:KCH[kc][1], 0:gw],
                                start=True, stop=True)
                            if kc == 0:
                                acc = cpool.tile([H1, 128], F32,
                                                 name=f"h1acc{e}",
                                                 tag=f"h1acc{e}")
                                h1acc.append(acc)
                                nc.vector.tensor_copy(acc[:, 0:gw],
                                                      h1p[:, 0:gw])
                            else:
                                nc.vector.tensor_tensor(
                                    out=h1acc[e][:, 0:gw],
                                    in0=h1acc[e][:, 0:gw], in1=h1p[:, 0:gw],
                                    op=mybir.AluOpType.add)
                    # now the stream-end-dependent work: kc3 transpose,
                    # gating kc3 + softmax, then the expert epilogues
                    xts.append(emit_chunk(3))
                    nc.tensor.matmul(glp[:, 0:gw], wg_sb[3][:],
                                     xts[3][0:KCH[3][1], 0:gw],
                                     start=False, stop=True,
                                     skip_group_check=True)
                    masked, gssb = finish_gating()
                    for e in range(E):
                        h1p = h1ppool.tile([H1, TGW], F32, space="PSUM",
                                           tag="h1p")
                        nc.tensor.matmul(
                            h1p[:, 0:gw], w1_sb[3][:, e * H1:(e + 1) * H1],
                            xts[3][0:KCH[3][1], 0:gw], start=True, stop=True)
                        h1f = h1spool.tile([H1, TGW], F32, tag="h1f")
                        nc.vector.tensor_tensor(
                            out=h1f[:, 0:gw], in0=h1acc[e][:, 0:gw],
                            in1=h1p[:, 0:gw], op=mybir.AluOpType.add)
                        h1s = h1spool.tile([H1, TGW], BF16, tag="h1s")
                        nc.scalar.activation(h1s[:, 0:gw], h1f[:, 0:gw],
                                             mybir.ActivationFunctionType.Relu,
                                             bias=b1_sb[:, e:e + 1])
                        h2p = h2ppool.tile([H2, TGW], F32, space="PSUM",
                                           tag="h2p")
                        nc.tensor.matmul(h2p[:, 0:gw],
                                         w2_sb[:, e * H2:(e + 1) * H2],
                                         h1s[:, 0:gw], start=True, stop=True)
                        h2s = h2spool.tile([H2, TGW], BF16, tag="h2s")
                        if use_b2:
                            h2a = h2spool.tile([H2, TGW], F32, tag="h2a")
                            nc.vector.tensor_tensor(
                                out=h2a[:, 0:gw], in0=h2p[:, 0:gw],
                                in1=b2_sb[:, e:e + 1].to_broadcast([H2, gw]),
                                op=mybir.AluOpType.add)
                            nc.vector.tensor_scalar_max(h2s[:, 0:gw],
                                                        h2a[:, 0:gw], 0.0)
                        else:
                            nc.vector.tensor_scalar_max(h2s[:, 0:gw],
                                                        h2p[:, 0:gw], 0.0)
                        nc.tensor.matmul(sp[:, 0:gw],
                                         wo_sb[:, e * 8:(e + 1) * 8],
                                         h2s[:, 0:gw],
                                         start=(e == 0), stop=(e == E - 1),
                                         skip_group_check=True)
                else:
                    for e in range(E):
                        h1p = h1ppool.tile([H1, TGW], F32, space="PSUM",
                                           tag="h1p")
                        for kc in range(len(KCH)):
                            nc.tensor.matmul(
                                h1p[:, 0:gw], w1_sb[kc][:, e * H1:(e + 1) * H1],
                                xts[kc][0:KCH[kc][1], 0:gw],
                                start=(kc == 0), stop=(kc == len(KCH) - 1))
                        h1s = h1spool.tile([H1, TGW], BF16, tag="h1s")
                        nc.scalar.activation(h1s[:, 0:gw], h1p[:, 0:gw],
                                             mybir.ActivationFunctionType.Relu,
                                             bias=b1_sb[:, e:e + 1])
                        h2p = h2ppool.tile([H2, TGW], F32, space="PSUM",
                                           tag="h2p")
                        nc.tensor.matmul(h2p[:, 0:gw],
                                         w2_sb[:, e * H2:(e + 1) * H2],
                                         h1s[:, 0:gw], start=True, stop=True)
                        # h2 bias+relu on DVE (keeps ACT off the tail path)
                        h2s = h2spool.tile([H2, TGW], BF16, tag="h2s")
                        if use_b2:
                            h2a = h2spool.tile([H2, TGW], F32, tag="h2a")
                            nc.vector.tensor_tensor(
                                out=h2a[:, 0:gw], in0=h2p[:, 0:gw],
                                in1=b2_sb[:, e:e + 1].to_broadcast([H2, gw]),
                                op=mybir.AluOpType.add)
                            nc.vector.tensor_scalar_max(h2s[:, 0:gw],
                                                        h2a[:, 0:gw], 0.0)
                        else:
                            nc.vector.tensor_scalar_max(h2s[:, 0:gw],
                                                        h2p[:, 0:gw], 0.0)
                        nc.tensor.matmul(sp[:, 0:gw],
                                         wo_sb[:, e * 8:(e + 1) * 8],
                                         h2s[:, 0:gw],
                                         start=(e == 0), stop=(e == E - 1),
                                         skip_group_check=True)

                # --- final: logits = (sum_e gsel*s)/denom + bo ---
                msb = finpool.tile([E, TGW], BF16, tag="msb")
                nc.vector.tensor_tensor(out=msb[:, 0:gw], in0=sp[:, 0:gw],
                                        in1=gssb[:, 0:gw],
                                        op=mybir.AluOpType.mult)
                updn = sppool.tile([E, 2 * TGW], F32, space="PSUM", tag="spp")
                nc.tensor.matmul(updn[0:1, 0:gw], on8_sb[:], msb[:, 0:gw],
                                 start=True, stop=True)
                nc.tensor.matmul(updn[0:1, TGW:TGW + gw], on32_sb[:],
                                 masked[:, 0:gw], start=True, stop=True)
                rr = finpool.tile([1, TGW], F32, tag="rr")
                nc.vector.reciprocal(rr[0:1, 0:gw], updn[0:1, TGW:TGW + gw])
                lsb = finpool.tile([1, TGW], F32, tag="lsb")
                nc.vector.tensor_tensor(out=lsb[0:1, 0:gw],
                                        in0=updn[0:1, 0:gw],
                                        in1=rr[0:1, 0:gw],
                                        op=mybir.AluOpType.mult)
                if bo_val != 0.0:
                    nc.vector.tensor_scalar_add(lsb[0:1, 0:gw],
                                                lsb[0:1, 0:gw], float(bo_val))
                nc.sync.dma_start(out=out[0:1, t0 * 128: t0 * 128 + gw],
                                  in_=lsb[0:1, 0:gw])

    nc.compile()
    return nc


def kernel(**inputs):
    features = np.asarray(inputs["features"])
    domain = np.asarray(inputs["domain_indicator"])
    emb = np.asarray(inputs["emb"], dtype=np.float32)
    W1 = np.asarray(inputs["W1"], dtype=np.float32)
    b1 = np.asarray(inputs["b1"], dtype=np.float32)
    W2 = np.asarray(inputs["W2"], dtype=np.float32)
    b2 = np.asarray(inputs["b2"], dtype=np.float32)
    Wg = np.asarray(inputs["Wg"], dtype=np.float32)
    bg = np.asarray(inputs["bg"], dtype=np.float32)
    Wo = np.asarray(inputs["Wo"], dtype=np.float32)
    bo = np.asarray(inputs["bo"], dtype=np.float32)

    bo_val = float(bo.reshape(-1)[0])
    use_b2 = bool(np.any(b2))
    key = ("m3oe", bo_val, use_b2)
    if key not in _cache:
        _cache[key] = _build(bo_val, use_b2)
    nc = _cache[key]

    # ---- host-side prep (shared across cores) ----
    emb_flat = np.ascontiguousarray(
        emb.reshape(F * V, DK)).astype(ml_dtypes.bfloat16)
    # flattened per-(sample,field) row index into emb_flat
    idx_all = (features.astype(np.int64) +
               (np.arange(F, dtype=np.int64) * V)[None, :]).astype(np.int32)

    w1k = []
    wgk = []
    for koff, kw in KCH:
        # w1k[i, e*H1+h] = W1[e, koff+i, h]
        w1k.append(np.ascontiguousarray(
            W1[:, koff:koff + kw, :].transpose(1, 0, 2).reshape(kw, E * H1)
        ).astype(ml_dtypes.bfloat16))
        wgk.append(np.ascontiguousarray(
            Wg[:, koff:koff + kw, :].transpose(1, 0, 2).reshape(kw, D * E)
        ).astype(ml_dtypes.bfloat16))
    w2c = np.ascontiguousarray(
        W2.transpose(1, 0, 2).reshape(H1, E * H2)).astype(ml_dtypes.bfloat16)
    # wo8[k, e*8+m] = Wo[k] iff m == e (so s_e lands on psum partition e)
    wo8 = np.zeros((H2, E * 8), dtype=ml_dtypes.bfloat16)
    wov = Wo.reshape(H2)
    for e in range(E):
        wo8[:, e * 8 + e] = wov
    sel8 = np.zeros((D * E, 8), dtype=ml_dtypes.bfloat16)
    for d in range(D):
        for e in range(E):
            sel8[d * 8 + e, e] = 1.0
    ones8 = np.ones((E, 1), dtype=ml_dtypes.bfloat16)
    ones32 = np.ones((D * E, 1), dtype=ml_dtypes.bfloat16)
    b1t = np.ascontiguousarray(b1.T)  # [H1, E]
    b2t = np.ascontiguousarray(b2.T)  # [H2, E]
    bgc = bg.reshape(D * E, 1).astype(np.float32)
    id128 = np.eye(128, dtype=ml_dtypes.bfloat16)

    shared = {
        "emb_flat": emb_flat, "w2c": w2c, "wo8": wo8, "sel8": sel8,
        "ones8": ones8, "ones32": ones32, "b1t": b1t, "b2t": b2t,
        "bgc": bgc, "id128": id128,
    }
    for i in range(len(KCH)):
        shared[f"w1k{i}"] = w1k[i]
        shared[f"wgk{i}"] = wgk[i]

    derep = np.repeat(np.arange(D), E)  # [32] domain of each (d,e) row
    in_maps = []
    for c in range(N_CORES):
        sl = slice(c * PC, (c + 1) * PC)
        # idx_core[p, t*F+f] = idx_all[c*PC + t*128 + p, f]
        idx_core = np.ascontiguousarray(
            idx_all[sl].reshape(NT, 128, F).transpose(1, 0, 2)
            .reshape(128, NT * F))
        dom = domain[sl].astype(np.int64)
        oh_core = (dom[None, :] == derep[:, None]).astype(np.float32)
        m = dict(shared)
        m["idx"] = idx_core
        m["oh"] = oh_core
        in_maps.append(m)

    global LAST_RESULT
    res = run_bass_kernel_spmd(nc, in_maps, core_ids=list(range(N_CORES)),
                               trace=TRACE, tmpdir=TRACE_TMPDIR)
    LAST_RESULT = res
    outs = [res.results[c]["out"].reshape(PC) for c in range(N_CORES)]
    return np.concatenate(outs).astype(np.float32)



# revision 28
# speedup vs baseline: 1.0082x; 1.0008x over previous
"""Trainium2 kernel for nn_M3oE: multi-domain MoE over 26 categorical embeddings.

Sharding: data-parallel over batch across 8 NeuronCores (2048 rows each),
embedding tables replicated in DRAM (gathered directly from HBM via
indirect DMA with inline f32->bf16 cast; tables never enter SBUF).

The kernel is gather-stream-bound: the SWDGE indirect DMA processes exactly
one offset per partition per instruction (HW-probed; multi-offset forms are
UB on the built-in Q7 ucode, and the custom gather ucode library is not
shipped in this image), so the 53248 embedding-row fetches per core cost
416 instructions at ~1.4us each (~1.09us Q7 ucode + ~0.31us issue gap)
= ~582us, regardless of batching. All compute hides under that stream; the
span beyond it is just NEFF startup (~10us) and the trailing compute,
which variable-size tile groups (4,4,4,2,1,1) keep small: the two 1-tile
tail groups defer their kc3 transpose + gating step and accumulate h1
k0..k2 partials in SBUF via DVE, so their bulk PE work overlaps the end
of the gather stream instead of queueing behind it.

Per-core pipeline (matmuls in bf16, PSUM f32):
  1. indirect-DMA gather of 26 rows/sample into xg [128, tiles*416] bf16
  2. PE transpose of x chunks -> xT [416(k-chunks), cols] (DVE evict, bf16)
  3. 8 experts: h1T = relu(W1^T xT + b1) (ACT evict fuses bias+relu+cast);
     h2T = relu(W2^T h1T + b2) (DVE add+max, off ACT's critical path);
     s_e = Wo . h2T accumulated into one [8, cols] PSUM tile via
     zero-padded Wo blocks
  4. domain-gated softmax: glog = Wg^T xT ; exp via ACT (+bg bias);
     one-hot domain mask (host-built) picks the sample's domain; expert-sum
     matmul (sel8) compresses to [8, cols]; denominator via ones32 matmul
  5. logits = (sum_e gsel_e * s_e) / denom + bo   (division via DVE recip)
"""

import ml_dtypes
import numpy as np

import concourse.bacc as bacc
import concourse.mybir as mybir
import concourse.tile as tile
from concourse.bass import IndirectOffsetOnAxis
from concourse.bass_utils import run_bass_kernel_spmd

F = 26
V = 100000
DK = 16
D = 4
E = 8
H1 = 128
H2 = 64
B = 16384
IN = F * DK  # 416
N_CORES = 8
PC = B // N_CORES  # 2048 rows per core
NT = PC // 128  # 16 batch-tiles of 128
NTG = 4  # tile groups (legacy; see GROUPS)
TGW = 512  # max columns per tile group (one PSUM bank of f32)
TPG = NT // NTG  # legacy
# Variable-size tile groups: big groups while the gather stream dominates,
# small trailing groups to shrink the post-stream compute tail.
GROUPS = [(0, 4), (4, 4), (8, 4), (12, 2), (14, 1), (15, 1)]
KCH = [(0, 128), (128, 128), (256, 128), (384, 32)]  # k-chunks of IN=416

F32 = mybir.dt.float32
F32R = mybir.dt.float32r
BF16 = mybir.dt.bfloat16
I32 = mybir.dt.int32

_cache = {}

# test-harness knobs (unused when the harness calls kernel() directly)
TRACE = False
TRACE_TMPDIR = None
LAST_RESULT = None


def _build(bo_val: float, use_b2: bool):
    nc = bacc.Bacc("TRN2", target_bir_lowering=False, debug=False,
                   num_devices=N_CORES)

    emb = nc.dram_tensor("emb_flat", [F * V, DK], BF16, kind="ExternalInput")
    idx = nc.dram_tensor("idx", [128, NT * F], I32, kind="ExternalInput")
    w1k = [nc.dram_tensor(f"w1k{i}", [w, E * H1], BF16, kind="ExternalInput")
           for i, (_, w) in enumerate(KCH)]
    w2c = nc.dram_tensor("w2c", [H1, E * H2], BF16, kind="ExternalInput")
    wo8 = nc.dram_tensor("wo8", [H2, E * 8], BF16, kind="ExternalInput")
    wgk = [nc.dram_tensor(f"wgk{i}", [w, D * E], BF16, kind="ExternalInput")
           for i, (_, w) in enumerate(KCH)]
    sel8 = nc.dram_tensor("sel8", [D * E, 8], BF16, kind="ExternalInput")
    ones8 = nc.dram_tensor("ones8", [E, 1], BF16, kind="ExternalInput")
    ones32 = nc.dram_tensor("ones32", [D * E, 1], BF16, kind="ExternalInput")
    b1t = nc.dram_tensor("b1t", [H1, E], F32, kind="ExternalInput")
    b2t = nc.dram_tensor("b2t", [H2, E], F32, kind="ExternalInput")
    bgc = nc.dram_tensor("bgc", [D * E, 1], F32, kind="ExternalInput")
    oh = nc.dram_tensor("oh", [D * E, PC], F32, kind="ExternalInput")
    id128 = nc.dram_tensor("id128", [128, 128], BF16, kind="ExternalInput")
    out = nc.dram_tensor("out", [1, PC], F32, kind="ExternalOutput")

    with tile.TileContext(nc) as tc:
        with (
            tc.tile_pool(name="const", bufs=1) as cpool,
            tc.tile_pool(name="xg", bufs=4) as xgpool,
            tc.tile_pool(name="xts", bufs=2 * len(KCH)) as xtspool,
            tc.tile_pool(name="h1s", bufs=3) as h1spool,
            tc.tile_pool(name="h2s", bufs=2) as h2spool,
            tc.tile_pool(name="gsb", bufs=2) as gsbpool,
            tc.tile_pool(name="fin", bufs=2) as finpool,
            tc.tile_pool(name="xtp", bufs=2, space="PSUM") as xtppool,
            tc.tile_pool(name="h1p", bufs=2, space="PSUM") as h1ppool,
            tc.tile_pool(name="h2p", bufs=1, space="PSUM") as h2ppool,
            tc.tile_pool(name="glp", bufs=1, space="PSUM") as glppool,
            tc.tile_pool(name="spp", bufs=1, space="PSUM") as sppool,
        ):
            # --- load constants ---
            def cload(dram, shape, dtype=None):
                t = cpool.tile(shape, dtype or dram.dtype, tag=dram.name)
                nc.sync.dma_start(out=t[:], in_=dram[:])
                return t

            idx_sb = cload(idx, [128, NT * F], I32)
            w1_sb = [cload(w1k[i], [w, E * H1]) for i, (_, w) in enumerate(KCH)]
            w2_sb = cload(w2c, [H1, E * H2])
            wo_sb = cload(wo8, [H2, E * 8])
            wg_sb = [cload(wgk[i], [w, D * E]) for i, (_, w) in enumerate(KCH)]
            sel_sb = cload(sel8, [D * E, 8])
            on8_sb = cload(ones8, [E, 1])
            on32_sb = cload(ones32, [D * E, 1])
            b1_sb = cload(b1t, [H1, E])
            b2_sb = cload(b2t, [H2, E])
            bg_sb = cload(bgc, [D * E, 1])
            oh_sb = cload(oh, [D * E, PC])
            id_sb = cload(id128, [128, 128])

            for t0, ntl in GROUPS:
                gw = ntl * 128  # columns in this group
                # --- gather: this group's embedding rows ---
                # SWDGE contract (HW-probed): one offset per partition per
                # instruction, offset read from the offset AP's first column
                # at each partition. F instructions per batch-tile is the
                # floor (~1.4us each: ~1.09us Q7 ucode + ~0.31us issue gap).
                xg = xgpool.tile([128, TPG * IN], BF16, tag="xg")
                if ntl == 1:
                    # gather kc3's two fields before kc2's eight so the
                    # cheap kc3 step can fold into the early accumulation
                    # and only kc2 trails the stream
                    gorder = list(range(16)) + [24, 25] + list(range(16, 24))
                else:
                    gorder = list(range(ntl * F))
                for j in gorder:
                    jj = t0 * F + j
                    nc.gpsimd.indirect_dma_start(
                        out=xg[:, j * DK:(j + 1) * DK],
                        out_offset=None,
                        in_=emb[:],
                        in_offset=IndirectOffsetOnAxis(
                            ap=idx_sb[:, jj:jj + 1], axis=0),
                    )

                # --- transpose x -> xT per k-chunk (PE), evict to SBUF ---
                # For the last 1-tile group, defer the kc3 transpose (which
                # waits on the very last gather) so it doesn't head-of-line
                # block the kc0-2 expert matmuls on the in-order PE queue.
                def emit_chunk(kc):
                    koff, kw = KCH[kc]
                    xtp = xtppool.tile([128, TGW], BF16, space="PSUM",
                                       name=f"xtp{kc}", tag="xtp")
                    for tl in range(ntl):
                        nc.tensor.transpose(
                            out=xtp[0:kw, tl * 128:(tl + 1) * 128],
                            in_=xg[:, tl * IN + koff: tl * IN + koff + kw],
                            identity=id_sb[:],
                        )
                    xt = xtspool.tile([kw, TGW], BF16, name=f"xt{kc}",
                                      tag=f"xts{kc}")
                    nc.vector.tensor_copy(xt[0:kw, 0:gw], xtp[0:kw, 0:gw])
                    return xt

                last = ntl == 1
                early = [0, 1, 3] if last else [0, 1, 2, 3]
                xts = {kc: emit_chunk(kc) for kc in early}

                # --- gating (kc2 step deferred for 1-tile groups) ---
                glp = glppool.tile([D * E, TGW], F32, space="PSUM", tag="glp")
                for kc in early:
                    nc.tensor.matmul(glp[:, 0:gw], wg_sb[kc][:],
                                     xts[kc][0:KCH[kc][1], 0:gw],
                                     start=(kc == 0),
                                     stop=(not last and kc == len(KCH) - 1),
                                     skip_group_check=True)

                def finish_gating():
                    expsb = gsbpool.tile([D * E, TGW], F32, tag="expsb")
                    nc.scalar.activation(expsb[:, 0:gw], glp[:, 0:gw],
                                         mybir.ActivationFunctionType.Exp,
                                         bias=bg_sb[:, 0:1])
                    masked = gsbpool.tile([D * E, TGW], BF16, tag="masked")
                    nc.vector.tensor_tensor(
                        out=masked[:, 0:gw], in0=expsb[:, 0:gw],
                        in1=oh_sb[:, t0 * 128: t0 * 128 + gw],
                        op=mybir.AluOpType.mult)
                    gslp = glppool.tile([D * E, TGW], F32, space="PSUM",
                                        tag="glp")
                    nc.tensor.matmul(gslp[0:8, 0:gw], sel_sb[:],
                                     masked[:, 0:gw], start=True, stop=True)
                    gssb = gsbpool.tile([8, TGW], F32, tag="gssb")
                    nc.scalar.activation(gssb[:, 0:gw], gslp[0:8, 0:gw],
                                         mybir.ActivationFunctionType.Copy)
                    return masked, gssb

                if not last:
                    masked, gssb = finish_gating()

                # --- experts ---
                sp = sppool.tile([E, 2 * TGW], F32, space="PSUM", tag="spp")
                if ntl == 1:
                    # Last tiny group: accumulate each expert's k0..k2 h1
                    # partials in SBUF (DVE) so the two h1 PSUM banks recycle
                    # and all 8 experts' bulk work runs during the gather
                    # stream; only the kc3 step + epilogues trail it.
                    h1acc = []
                    for kc in early:
                        for e in range(E):
                            h1p = h1ppool.tile([H1, TGW], F32, space="PSUM",
                                               tag="h1p")
                            nc.tensor.matmul(
                                h1p[:, 0:gw],
                                w1_sb[kc][:, e * H1:(e + 1) * H1],
                                xts[kc][0:KCH[kc][1], 0:gw],
                                start=True, stop=True)
                            if kc == 0:
                                acc = cpool.tile([H1, 128], F32,
                                                 name=f"h1acc{e}",
                                                 tag=f"h1acc{e}")
                                h1acc.append(acc)
                                nc.vector.tensor_copy(acc[:, 0:gw],
                                                      h1p[:, 0:gw])
                            else:
                                nc.vector.tensor_tensor(
                                    out=h1acc[e][:, 0:gw],
                                    in0=h1acc[e][:, 0:gw], in1=h1p[:, 0:gw],
                                    op=mybir.AluOpType.add)
                    # now the stream-end-dependent work: kc2 transpose,
                    # gating kc2 + softmax, then the expert epilogues
                    xts[2] = emit_chunk(2)
                    nc.tensor.matmul(glp[:, 0:gw], wg_sb[2][:],
                                     xts[2][0:KCH[2][1], 0:gw],
                                     start=False, stop=True,
                                     skip_group_check=True)
                    masked, gssb = finish_gating()
                    for e in range(E):
                        h1p = h1ppool.tile([H1, TGW], F32, space="PSUM",
                                           tag="h1p")
                        nc.tensor.matmul(
                            h1p[:, 0:gw], w1_sb[2][:, e * H1:(e + 1) * H1],
                            xts[2][0:KCH[2][1], 0:gw], start=True, stop=True)
                        h1f = h1spool.tile([H1, TGW], F32, tag="h1f")
                        nc.vector.tensor_tensor(
                            out=h1f[:, 0:gw], in0=h1acc[e][:, 0:gw],
                            in1=h1p[:, 0:gw], op=mybir.AluOpType.add)
                        h1s = h1spool.tile([H1, TGW], BF16, tag="h1s")
                        nc.scalar.activation(h1s[:, 0:gw], h1f[:, 0:gw],
                                             mybir.ActivationFunctionType.Relu,
                                             bias=b1_sb[:, e:e + 1])
                        h2p = h2ppool.tile([H2, TGW], F32, space="PSUM",
                                           tag="h2p")
                        nc.tensor.matmul(h2p[:, 0:gw],
                                         w2_sb[:, e * H2:(e + 1) * H2],
                                         h1s[:, 0:gw], start=True, stop=True)
                        h2s = h2spool.tile([H2, TGW], BF16, tag="h2s")
                        if use_b2:
                            h2a = h2spool.tile([H2, TGW], F32, tag="h2a")
                            nc.vector.tensor_tensor(
                                out=h2a[:, 0:gw], in0=h2p[:, 0:gw],
                                in1=b2_sb[:, e:e + 1].to_broadcast([H2, gw]),
                                op=mybir.AluOpType.add)
                            nc.vector.tensor_scalar_max(h2s[:, 0:gw],
                                                        h2a[:, 0:gw], 0.0)
                        else:
                            nc.vector.tensor_scalar_max(h2s[:, 0:gw],
                                                        h2p[:, 0:gw], 0.0)
                        nc.tensor.matmul(sp[:, 0:gw],
                                         wo_sb[:, e * 8:(e + 1) * 8],
                                         h2s[:, 0:gw],
                                         start=(e == 0), stop=(e == E - 1),
                                         skip_group_check=True)
                else:
                    for e in range(E):
                        h1p = h1ppool.tile([H1, TGW], F32, space="PSUM",
                                           tag="h1p")
                        for kc in range(len(KCH)):
                            nc.tensor.matmul(
                                h1p[:, 0:gw], w1_sb[kc][:, e * H1:(e + 1) * H1],
                                xts[kc][0:KCH[kc][1], 0:gw],
                                start=(kc == 0), stop=(kc == len(KCH) - 1))
                        h1s = h1spool.tile([H1, TGW], BF16, tag="h1s")
                        nc.scalar.activation(h1s[:, 0:gw], h1p[:, 0:gw],
                                             mybir.ActivationFunctionType.Relu,
                                             bias=b1_sb[:, e:e + 1])
                        h2p = h2ppool.tile([H2, TGW], F32, space="PSUM",
                                           tag="h2p")
                        nc.tensor.matmul(h2p[:, 0:gw],
                                         w2_sb[:, e * H2:(e + 1) * H2],
                                         h1s[:, 0:gw], start=True, stop=True)
                        # h2 bias+relu on DVE (keeps ACT off the tail path)
                        h2s = h2spool.tile([H2, TGW], BF16, tag="h2s")
                        if use_b2:
                            h2a = h2spool.tile([H2, TGW], F32, tag="h2a")
                            nc.vector.tensor_tensor(
                                out=h2a[:, 0:gw], in0=h2p[:, 0:gw],
                                in1=b2_sb[:, e:e + 1].to_broadcast([H2, gw]),
                                op=mybir.AluOpType.add)
                            nc.vector.tensor_scalar_max(h2s[:, 0:gw],
                                                        h2a[:, 0:gw], 0.0)
                        else:
                            nc.vector.tensor_scalar_max(h2s[:, 0:gw],
                                                        h2p[:, 0:gw], 0.0)
                        nc.tensor.matmul(sp[:, 0:gw],
                                         wo_sb[:, e * 8:(e + 1) * 8],
                                         h2s[:, 0:gw],
                                         start=(e == 0), stop=(e == E - 1),
                                         skip_group_check=True)

                # --- final: logits = (sum_e gsel*s)/denom + bo ---
                msb = finpool.tile([E, TGW], BF16, tag="msb")
                nc.vector.tensor_tensor(out=msb[:, 0:gw], in0=sp[:, 0:gw],
                                        in1=gssb[:, 0:gw],
                                        op=mybir.AluOpType.mult)
                updn = sppool.tile([E, 2 * TGW], F32, space="PSUM", tag="spp")
                nc.tensor.matmul(updn[0:1, 0:gw], on8_sb[:], msb[:, 0:gw],
                                 start=True, stop=True)
                nc.tensor.matmul(updn[0:1, TGW:TGW + gw], on32_sb[:],
                                 masked[:, 0:gw], start=True, stop=True)
                rr = finpool.tile([1, TGW], F32, tag="rr")
                nc.vector.reciprocal(rr[0:1, 0:gw], updn[0:1, TGW:TGW + gw])
                lsb = finpool.tile([1, TGW], F32, tag="lsb")
                nc.vector.tensor_tensor(out=lsb[0:1, 0:gw],
                                        in0=updn[0:1, 0:gw],
                                        in1=rr[0:1, 0:gw],
                                        op=mybir.AluOpType.mult)
                if bo_val != 0.0:
                    nc.vector.tensor_scalar_add(lsb[0:1, 0:gw],
                                                lsb[0:1, 0:gw], float(bo_val))
                nc.sync.dma_start(out=out[0:1, t0 * 128: t0 * 128 + gw],
                                  in_=lsb[0:1, 0:gw])

    nc.compile()
    return nc


def kernel(**inputs):
    features = np.asarray(inputs["features"])
    domain = np.asarray(inputs["domain_indicator"])
    emb = np.asarray(inputs["emb"], dtype=np.float32)
    W1 = np.asarray(inputs["W1"], dtype=np.float32)
    b1 = np.asarray(inputs["b1"], dtype=np.float32)
    W2 = np.asarray(inputs["W2"], dtype=np.float32)
    b2 = np.asarray(inputs["b2"], dtype=np.float32)
    Wg = np.asarray(inputs["Wg"], dtype=np.float32)
    bg = np.asarray(inputs["bg"], dtype=np.float32)
    Wo = np.asarray(inputs["Wo"], dtype=np.float32)
    bo = np.asarray(inputs["bo"], dtype=np.float32)

    bo_val = float(bo.reshape(-1)[0])
    use_b2 = bool(np.any(b2))
    key = ("m3oe", bo_val, use_b2)
    if key not in _cache:
        _cache[key] = _build(bo_val, use_b2)
    nc = _cache[key]

    # ---- host-side prep (shared across cores) ----
    emb_flat = np.ascontiguousarray(
        emb.reshape(F * V, DK)).astype(ml_dtypes.bfloat16)
    # flattened per-(sample,field) row index into emb_flat
    idx_all = (features.astype(np.int64) +
               (np.arange(F, dtype=np.int64) * V)[None, :]).astype(np.int32)

    w1k = []
    wgk = []
    for koff, kw in KCH:
        # w1k[i, e*H1+h] = W1[e, koff+i, h]
        w1k.append(np.ascontiguousarray(
            W1[:, koff:koff + kw, :].transpose(1, 0, 2).reshape(kw, E * H1)
        ).astype(ml_dtypes.bfloat16))
        wgk.append(np.ascontiguousarray(
            Wg[:, koff:koff + kw, :].transpose(1, 0, 2).reshape(kw, D * E)
        ).astype(ml_dtypes.bfloat16))
    w2c = np.ascontiguousarray(
        W2.transpose(1, 0, 2).reshape(H1, E * H2)).astype(ml_dtypes.bfloat16)
    # wo8[k, e*8+m] = Wo[k] iff m == e (so s_e lands on psum partition e)
    wo8 = np.zeros((H2, E * 8), dtype=ml_dtypes.bfloat16)
    wov = Wo.reshape(H2)
    for e in range(E):
        wo8[:, e * 8 + e] = wov
    sel8 = np.zeros((D * E, 8), dtype=ml_dtypes.bfloat16)
    for d in range(D):
        for e in range(E):
            sel8[d * 8 + e, e] = 1.0
    ones8 = np.ones((E, 1), dtype=ml_dtypes.bfloat16)
    ones32 = np.ones((D * E, 1), dtype=ml_dtypes.bfloat16)
    b1t = np.ascontiguousarray(b1.T)  # [H1, E]
    b2t = np.ascontiguousarray(b2.T)  # [H2, E]
    bgc = bg.reshape(D * E, 1).astype(np.float32)
    id128 = np.eye(128, dtype=ml_dtypes.bfloat16)

    shared = {
        "emb_flat": emb_flat, "w2c": w2c, "wo8": wo8, "sel8": sel8,
        "ones8": ones8, "ones32": ones32, "b1t": b1t, "b2t": b2t,
        "bgc": bgc, "id128": id128,
    }
    for i in range(len(KCH)):
        shared[f"w1k{i}"] = w1k[i]
        shared[f"wgk{i}"] = wgk[i]

    derep = np.repeat(np.arange(D), E)  # [32] domain of each (d,e) row
    in_maps = []
    for c in range(N_CORES):
        sl = slice(c * PC, (c + 1) * PC)
        # idx_core[p, t*F+f] = idx_all[c*PC + t*128 + p, f]
        idx_core = np.ascontiguousarray(
            idx_all[sl].reshape(NT, 128, F).transpose(1, 0, 2)
            .reshape(128, NT * F))
        dom = domain[sl].astype(np.int64)
        oh_core = (dom[None, :] == derep[:, None]).astype(np.float32)
        m = dict(shared)
        m["idx"] = idx_core
        m["oh"] = oh_core
        in_maps.append(m)

    global LAST_RESULT
    res = run_bass_kernel_spmd(nc, in_maps, core_ids=list(range(N_CORES)),
                               trace=TRACE, tmpdir=TRACE_TMPDIR)
    LAST_RESULT = res
    outs = [res.results[c]["out"].reshape(PC) for c in range(N_CORES)]
    return np.concatenate(outs).astype(np.float32)

